# revision 19
# baseline (speedup 1.0000x reference)
"""Multi-head attention (B=4, N=2048, C=1024, H=16, Dh=64) on 8 TRN2 NeuronCores.

Sharding: core c handles batch b=c//2 and head-group hg=c%2 (8 heads each).
Host pre-transposes / pre-casts inputs to bf16 so the device needs no
transposes; each core returns a partial projection output [2048, 1024] f32
(its 8 heads' contribution); the host sums core pairs and adds the bias.

Schedule: QKV prelude (first weight tile ct-major to overlap input DMA),
then per (query-chunk, head) slots where the AV matmuls of the previous
head are interleaved between the scores matmuls of the current head so the
ScalarE exp stream never starves; the output projection is spread into the
second query-chunk's early slots.
"""

from contextlib import ExitStack

import numpy as np
import ml_dtypes

B, N, C = 4, 2048, 1024
H, DH = 16, 64
NCORES = 8
P = 128
BF16 = ml_dtypes.bfloat16

_NC_CACHE = {}


def _build_nc():
    import concourse.bass as bass  # noqa: F401
    import concourse.tile as tile
    from concourse import bacc, mybir

    bf = mybir.dt.bfloat16
    f32 = mybir.dt.float32
    Exp = mybir.ActivationFunctionType.Exp

    nc = bacc.Bacc("TRN2", target_bir_lowering=False, debug=False,
                   num_devices=NCORES)

    xT_d = nc.dram_tensor("xT", [C, N], bf, kind="ExternalInput").ap()
    wqkvT_d = nc.dram_tensor("wqkvT", [C, 1536], bf, kind="ExternalInput").ap()
    wpT_d = nc.dram_tensor("wpT", [512, C], bf, kind="ExternalInput").ap()
    out_d = nc.dram_tensor("out", [N, C], bf, kind="ExternalOutput").ap()

    n_ct = C // P          # 8 contraction tiles over C
    n_tt = N // P          # 16 token tiles
    HL = H // 2            # 8 local heads

    with tile.TileContext(nc) as tc, ExitStack() as st:
        qk_pool = st.enter_context(tc.tile_pool(name="qk", bufs=8))
        v_pool = st.enter_context(tc.tile_pool(name="v1", bufs=16))
        wp_pool = st.enter_context(tc.tile_pool(name="wp", bufs=4))
        ot_pool = st.enter_context(tc.tile_pool(name="ot", bufs=4))
        misc_pool = st.enter_context(tc.tile_pool(name="misc", bufs=1))
        y_pool = st.enter_context(tc.tile_pool(name="y", bufs=3))
        den_pool = st.enter_context(tc.tile_pool(name="den", bufs=3))
        bc_pool = st.enter_context(tc.tile_pool(name="bc", bufs=3))
        pss_pool = st.enter_context(
            tc.tile_pool(name="ps_s", bufs=2, space="PSUM"))
        gen_pool = st.enter_context(
            tc.tile_pool(name="gen", bufs=4, space="PSUM"))

        wp = []
        for kt in range(4):
            w = wp_pool.tile([P, C], bf, name=f"wp{kt}", tag="wp")
            nc.sync.dma_start(w[:], wpT_d[kt * P:(kt + 1) * P, :])
            wp.append(w)

        # ---------------- prelude: load x/W, all QKV projections ----------
        qk = [None] * 8    # 0-3: qT row-tiles, 4-7: kT row-tiles
        v1 = [None] * n_tt
        outT = [ot_pool.tile([P, N], bf, name=f"outT{kt}", tag="ot")
                for kt in range(4)]

        load_pool = st.enter_context(tc.tile_pool(name="load", bufs=8))
        xts, wts = [], []
        for ct in range(n_ct):
            w = load_pool.tile([P, 1536], bf, name=f"wt{ct}", tag="wt")
            nc.sync.dma_start(w[:], wqkvT_d[ct * P:(ct + 1) * P, :])
            wts.append(w)
            x = load_pool.tile([P, N], bf, name=f"xt{ct}", tag="xt")
            nc.sync.dma_start(x[:], xT_d[ct * P:(ct + 1) * P, :])
            xts.append(x)

        def wcol(i):
            return i * P if i < 4 else 512 + (i - 4) * P

        def qk_chain(i, tc2, half):
            """One accumulation chain (8 matmuls + copy) of qk[i]."""
            col = tc2 * 1024 + half * 512
            ps = gen_pool.tile([P, 512], f32, name="ps_g", tag="gen")
            for ct in range(n_ct):
                nc.tensor.matmul(
                    ps[:],
                    wts[ct][:, wcol(i):wcol(i) + P],
                    xts[ct][:, col:col + 512],
                    start=(ct == 0), stop=(ct == n_ct - 1),
                    skip_group_check=True)
            nc.vector.tensor_copy(qk[i][:, col:col + 512], ps[:])

        def v_chain(tt):
            ps = gen_pool.tile([P, 512], f32, name="ps_g", tag="gen")
            for ct in range(n_ct):
                nc.tensor.matmul(ps[:],
                                 xts[ct][:, tt * P:(tt + 1) * P],
                                 wts[ct][:, 1024:1536],
                                 start=(ct == 0), stop=(ct == n_ct - 1),
                                 skip_group_check=True)
            vt = v_pool.tile([P, HL, 65], bf, name=f"v1_{tt}", tag="v1")
            nc.vector.tensor_copy(vt[:, :, 0:64],
                                  ps.rearrange("p (h d) -> p h d", d=64))
            nc.gpsimd.memset(vt[:, :, 64:65], 1.0)
            v1[tt] = vt

        # k4 (4 gen chains) + q0 (2 ps_s chains) together, ct-major, so the
        # PE rides the input-DMA stream with ~6 accumulation chains in flight
        tk = qk_pool.tile([P, N], bf, name="qk4", tag="qk")
        tq = qk_pool.tile([P, N], bf, name="qk0", tag="qk")
        k_ps = {ch: gen_pool.tile([P, 512], f32, name="ps_g", tag="gen")
                for ch in [(tc2, half) for tc2 in range(2) for half in range(2)]}
        q_ps = {tc2: pss_pool.tile([P, 1024], f32, name="ps_s", tag="ps_s")
                for tc2 in range(2)}
        for ct in range(n_ct):
            for (tc2, half), ps in k_ps.items():
                col = tc2 * 1024 + half * 512
                nc.tensor.matmul(ps[:], wts[ct][:, wcol(4):wcol(4) + P],
                                 xts[ct][:, col:col + 512],
                                 start=(ct == 0), stop=(ct == n_ct - 1),
                                 skip_group_check=True)
            for tc2, ps in q_ps.items():
                for half in range(2):
                    col = tc2 * 1024 + half * 512
                    nc.tensor.matmul(ps[:, half * 512:(half + 1) * 512],
                                     wts[ct][:, wcol(0):wcol(0) + P],
                                     xts[ct][:, col:col + 512],
                                     start=(ct == 0), stop=(ct == n_ct - 1),
                                     skip_group_check=True)
        for (tc2, half), ps in k_ps.items():
            col = tc2 * 1024 + half * 512
            nc.vector.tensor_copy(tk[:, col:col + 512], ps[:])
        for tc2, ps in q_ps.items():
            nc.vector.tensor_copy(tq[:, tc2 * 1024:(tc2 + 1) * 1024], ps[:])
        qk[4], qk[0] = tk, tq

        for i in (5, 1):
            qk[i] = qk_pool.tile([P, N], bf, name=f"qk{i}", tag="qk")
            for tc2 in range(2):
                for half in range(2):
                    qk_chain(i, tc2, half)
        for tt in range(n_tt):
            v_chain(tt)
        for i in (6, 2, 7, 3):
            qk[i] = qk_pool.tile([P, N], bf, name=f"qk{i}", tag="qk")

        et_pool = st.enter_context(tc.tile_pool(name="et", bufs=8))

        # ---------------- attention slots + projection --------------------
        def division(pv):
            """Divide po[0:64] by the ones-column denominator, write outT."""
            dr, r0 = pv["h"] // 2, (pv["h"] % 2) * 64
            for k in range(2):
                po = pv["po"][k]
                m0 = pv["mc2"] * 1024 + k * 512
                den = den_pool.tile([1, 512], f32, name="den", tag="den")
                nc.vector.tensor_copy(den[:], po[64:65, :])
                rec = den_pool.tile([1, 512], f32, name="rec", tag="rec")
                nc.vector.reciprocal(rec[:], den[:])
                bc = bc_pool.tile([64, 512], f32, name="bc", tag="bc")
                nc.gpsimd.partition_broadcast(bc[:], rec[:])
                nc.vector.tensor_mul(outT[dr][r0:r0 + 64, m0:m0 + 512],
                                     po[0:64, :], bc[:])

        def proj_unit(tt, oc):
            py = gen_pool.tile([P, 512], f32, name="ps_y", tag="gen")
            for kt in range(4):
                nc.tensor.matmul(py[:],
                                 outT[kt][:, tt * P:(tt + 1) * P],
                                 wp[kt][:, oc * 512:(oc + 1) * 512],
                                 start=(kt == 0), stop=(kt == 3),
                                 skip_group_check=True)
            y = y_pool.tile([P, 512], bf, name="yt", tag="y")
            nc.vector.tensor_copy(y[:], py[:])
            nc.sync.dma_start(
                out_d[tt * P:(tt + 1) * P, oc * 512:(oc + 1) * 512], y[:])

        # Continuous stream: one scores matmul pair + exp per iteration; the
        # matching AV pair trails two iterations behind via a global queue
        # (crossing slot boundaries without a bubble), with the division
        # riding the queue after each head's last AV pair.  Deferred QKV
        # chains and the chunk-0 projection are sprinkled in as extra PE
        # work, meeting the consumer-head deadlines (k6/q2 by slot 4 of
        # chunk 0, k7/q3 by slot 6).
        defer_chains = [(i, tc2, half) for i in (6, 2, 7, 3)
                        for tc2 in range(2) for half in range(2)]
        av_q = []

        def drain_av(k=1):
            for _ in range(k):
                if av_q:
                    av_q.pop(0)()

        for mc2 in range(2):
            for h in range(HL):
                dr, r0 = h // 2, (h % 2) * 64
                extras = []
                if mc2 == 0 and h < 6:
                    lo, hi = 16 * h // 6, 16 * (h + 1) // 6
                    extras = [(lambda a=a: qk_chain(*a))
                              for a in defer_chains[lo:hi]]
                if mc2 == 1 and 1 <= h <= 4:
                    units = [(tt, oc) for tt in range(8) for oc in range(2)]
                    extras = [(lambda u=u: proj_unit(*u))
                              for u in units[(h - 1) * 4: h * 4]]
                pv = {"h": h, "mc2": mc2,
                      "po": [gen_pool.tile([65, 512], f32,
                                           name="ps_o", tag="gen")
                             for _ in range(2)],
                      "ets": [None] * n_tt}
                for jt in range(n_tt):
                    ps = pss_pool.tile([P, 1024], f32, name="ps_s",
                                       tag="ps_s")
                    for half in range(2):
                        m0 = mc2 * 1024 + half * 512
                        nc.tensor.matmul(
                            ps[:, half * 512:(half + 1) * 512],
                            qk[4 + dr][r0:r0 + 64, jt * P:(jt + 1) * P],
                            qk[dr][r0:r0 + 64, m0:m0 + 512],
                            start=True, stop=True, skip_group_check=True)
                    et = et_pool.tile([P, 1024], bf, name="et", tag="et")
                    nc.scalar.activation(et[:], ps[:], Exp, scale=DH ** -0.5)
                    pv["ets"][jt] = et

                    def av_pair(pv=pv, jt=jt):
                        for k in range(2):
                            nc.tensor.matmul(
                                pv["po"][k][:],
                                v1[jt][:, pv["h"], :],
                                pv["ets"][jt][:, k * 512:(k + 1) * 512],
                                start=(jt == 0), stop=(jt == n_tt - 1),
                                skip_group_check=True)
                        if jt == n_tt - 1:
                            division(pv)
                    av_q.append(av_pair)
                    if len(av_q) > 2:
                        drain_av()
                    if extras and jt % 5 == 1:
                        extras.pop(0)()
                while extras:
                    extras.pop(0)()

        drain_av(len(av_q))

        # tail: projection of chunk 1
        for tt in range(8, 16):
            for oc in range(2):
                proj_unit(tt, oc)

    nc.compile()
    return nc


def get_nc():
    if "nc" not in _NC_CACHE:
        _NC_CACHE["nc"] = _build_nc()
    return _NC_CACHE["nc"]


def make_in_maps(x, W_qkv, W_proj):
    """Per-core bf16 pre-transposed shards (softmax scale folded into exp)."""
    xT = [np.ascontiguousarray(x[b].T).astype(BF16) for b in range(B)]
    in_maps = []
    for c in range(NCORES):
        b, hg = c // 2, c % 2
        r = slice(hg * 512, (hg + 1) * 512)
        wq = W_qkv[0:1024][r]
        wk = W_qkv[1024:2048][r]
        wv = W_qkv[2048:3072][r]
        wqkvT = np.ascontiguousarray(
            np.concatenate([wq, wk, wv], axis=0).T).astype(BF16)
        wpT = np.ascontiguousarray(W_proj[:, r].T).astype(BF16)
        in_maps.append({"xT": xT[b], "wqkvT": wqkvT, "wpT": wpT})
    return in_maps


LAST_RESULT = {}


def kernel(x, W_qkv, W_proj, b_proj):
    import os
    from concourse.bass_utils import run_bass_kernel_spmd

    nc = get_nc()
    in_maps = make_in_maps(np.asarray(x, dtype=np.float32),
                           np.asarray(W_qkv, dtype=np.float32),
                           np.asarray(W_proj, dtype=np.float32))
    trace = bool(int(os.environ.get("KERNEL_TRACE", "0")))
    try:
        res = run_bass_kernel_spmd(nc, in_maps, core_ids=list(range(NCORES)),
                                   trace=trace)
    except ModuleNotFoundError:
        res = run_bass_kernel_spmd(nc, in_maps, core_ids=list(range(NCORES)),
                                   trace=False)
    LAST_RESULT["exec_time_ns"] = res.exec_time_ns
    LAST_RESULT["res"] = res
    parts = [np.asarray(res.results[c]["out"], dtype=np.float32)
             for c in range(NCORES)]
    bp = np.asarray(b_proj, dtype=np.float32)
    out = np.stack([parts[2 * b] + parts[2 * b + 1] + bp for b in range(B)])
    return out.astype(np.float32)
